# revision 38
# baseline (speedup 1.0000x reference)
"""GCN 2-layer encoder on 8 Trainium2 NeuronCores (Bass/Tile).

kernel(**inputs) takes the FULL inputs and returns the FULL [80000, 32] f32
output.  Strategy (node partition across 8 cores, per sharding hint), ONE
fused SPMD launch with chunked in-kernel AllGathers:

  gcn_conv(x, W, b) = b + dinv * (A_hat @ (dinv * (x @ W)))  with self-loops,
  where dinv = 1/sqrt(indeg+1) and A_hat is the (unnormalized) adjacency.

  Both z tables are RANK-ordered (the host permutes x tiles by tile_of), so
  the two layers share ONE gather metadata set, and each dst tile's
  self-loop rows sit at a per-rank-uniform LOCAL offset (SPMD-identical
  across cores) -- they are lifted out of the gather into one contiguous
  DMA + identity matmuls per tile.

  z tables are stored as fp16 (hi | lo) pairs in 256B rows, lo = z - f32(hi):
  dma_gather moves 256B per row regardless (descriptor-bound, ~2.4ns/row on
  4 SWDGE queues), so the lo half rides for free and TWO accumulating fp16
  matmuls per 128-slot chunk recover full f32 precision at the fp16 PE rate
  (4x the f32 rate).

  Phase A: z1 = dinvS * (x @ W1) -> hi|lo -> ag1_in   (rank-ordered shard)
  AllGather(z1) in 3 chunks of 27/27/25 tile-ranks; chunk k fires as soon
           as phase A finishes those ranks; each chunk is one int16 gather
           range (<= 27648 rows).
  Phase B: per dst tile, gather 256B rows by edge source (gpsimd dma_gather,
           calls of <=1024 rows grouped over 4 dst tiles), reduce via
           one-hot scatter-matmul (lhsT=S8 fp16, rhs=msgs hi/lo) into PSUM;
           self rows via direct DMA + identity matmul; epilogue
           z2 = relu(dinv*agg + b1) * dinv -> hi|lo -> ag2_in
           (relu on the otherwise-idle ACT engine, hi/lo split on DVE+ACT).
  AllGather(z2) in 3 chunks (same overlap).
  Phase C: same gathers over z2 but the scatter-matmul runs with SWAPPED
           operands (lhsT=messages, rhs=one-hot) accumulating the
           TRANSPOSED aggregate accT[64,128], which feeds the final
           out = dinv*(accT^T @ W2) + b2 matmul directly per tile (no PE
           transpose pass, no separate final phase).  Output fp16 (host
           casts back; tolerance 2e-2 >> fp16 eps).

  Edge slots are grouped by (dst-tile group, src chunk, dst tile) with
  counts padded to a 32-multiple of the global per-rank max so all 8 cores
  run one identical SPMD program (PE operand partition windows allow bases
  0/32/64, so a 128-slot chunk straddling two tiles is reduced by
  partition-sliced matmul spans); gather indices sorted by source address
  for HBM locality.

  ALL inputs are packed into ONE f32 dram tensor (idx/dstv/iota ride as
  int16/fp16 bitcast regions): the per-launch dispatch floor scales with
  arg count (~56us/arg), so 13 args -> 2 is worth ~600us/launch.  Static
  DMAs are spread across both HWDGE queues (SP + ACT engines).
"""
import sys

sys.path.insert(0, '/opt/trn_rl_repo')

import numpy as np
import jax
from jax.sharding import Mesh, PartitionSpec
from jax.experimental.shard_map import shard_map

import concourse.bass as bass
import concourse.bacc as bacc
import concourse.tile as tile
import concourse.mybir as mybir
from concourse import bass2jax
from concourse.bass2jax import _bass_exec_p, partition_id_tensor
from concourse.masks import make_identity

F32 = mybir.dt.float32
BF16 = mybir.dt.bfloat16
FP16 = mybir.dt.float16
I16 = mybir.dt.int16

N_NODES = 80000
IN_CH = 128
HID = 64
OUT_CH = 32
N_CORES = 8
NT = N_NODES // 128                     # 625 dst tiles
TPC = (NT + N_CORES - 1) // N_CORES     # 79 tile ranks per core
ROWS = TPC * 128                        # 10112 rows per core
CH_TILES = [27, 27, 25]                 # AllGather chunking (tiles per chunk)
CH_START = [0, 27, 54]
RK = [t * 128 for t in CH_TILES]        # rows per core per chunk
R = 3                                   # one gather range per chunk
G = 4                                   # dst tiles per gather-call group
NG = (TPC + G - 1) // G
CALL_MAX = 1024                         # max indices per dma_gather call
SB = 16                                 # one-hot S matrices built per batch
MBUFS = 20                              # msgs pool double-buffer depth


def _ceil32(x):
    return ((x + 31) // 32) * 32


def _layer_pre(r_id, ridx, d_all, core_of_tile, tile_of, rank_of_tile):
    """Gather/scatter metadata (shared by both layers).

    Slot stream: segments (tile, range) in (group, range, tile) order,
    each padded only to a 32 multiple of the max-over-cores edge count
    (PE matmul partition bases must be 0/32/64/96).  Calls split the
    stream at <=CALL_MAX; a call-relative 128-chunk may straddle two
    segments -> per-chunk SPANS (tile, p0, p1, first, last) drive
    partition-sliced matmuls.
    """
    tile_g = d_all >> 7
    cnt_tile_r = np.bincount(tile_g * R + r_id,
                             minlength=NT * R).reshape(NT, R)
    cntK = np.zeros((N_CORES, TPC, R), np.int64)
    for c in range(N_CORES):
        real = tile_of[c][tile_of[c] >= 0]
        cntK[c, :len(real)] = cnt_tile_r[real]
    K = _ceil32(cntK.max(axis=0))       # [TPC, R] padded slot counts
    # PE matmul partition bases must be 0/32/64 (96 is rejected): bump any
    # segment whose cumulative in-stream boundary would land at 96 mod 128
    for g in range(NG):
        ts = list(range(g * G, min((g + 1) * G, TPC)))
        for r in range(R):
            cum = 0
            for t in ts:
                cum += int(K[t, r])
                if cum % 128 == 96:
                    K[t, r] += 32
                    cum += 32

    # segment order: (group, range, tile-in-group)
    seg_order = []
    for g in range(NG):
        ts = list(range(g * G, min((g + 1) * G, TPC)))
        for r in range(R):
            for t in ts:
                seg_order.append(t * R + r)
    seg_order = np.asarray(seg_order)
    seg_pos = np.empty(TPC * R, np.int64)
    seg_pos[seg_order] = np.arange(TPC * R)
    Kflat = K.reshape(-1)
    Koff_ord = np.zeros(TPC * R + 1, np.int64)
    Koff_ord[1:] = np.cumsum(Kflat[seg_order])
    T_pad = int(Koff_ord[-1])
    seg_off = np.empty(TPC * R, np.int64)
    seg_off[seg_order] = Koff_ord[:-1]

    # pad slots gather *scattered* rows (same-row reads serialize on one
    # HBM bank); dstl=-1 keeps them out of the one-hot reduction
    rngpad = np.random.default_rng(12345)
    sizes = np.asarray([N_CORES * r for r in RK])
    gidx_all = np.empty((N_CORES, T_pad), np.int16)
    for r in range(R):
        for g in range(NG):
            ts = list(range(g * G, min((g + 1) * G, TPC)))
            lo = int(seg_off[ts[0] * R + r])
            hi = lo + int(sum(K[t, r] for t in ts))
            gidx_all[:, lo:hi] = rngpad.integers(
                0, sizes[r], (N_CORES, hi - lo)).astype(np.int16)
    dstl_all = np.full((N_CORES, T_pad), -1.0, np.float32)
    ecore = core_of_tile[tile_g]
    for c in range(N_CORES):
        m = ecore == c
        e_rank = rank_of_tile[tile_g[m]]
        e_r = r_id[m]
        seg = e_rank * R + e_r
        sp = ridx[m]
        order = np.lexsort((sp, seg_pos[seg]))
        seg_s = seg[order]                      # sorted by segment ordinal
        seg_counts = np.bincount(seg_s, minlength=TPC * R)
        starts = np.zeros(TPC * R, np.int64)    # seg id -> stream start
        csum = np.cumsum(seg_counts[seg_order])
        starts[seg_order[1:]] = csum[:-1]
        within = np.arange(len(seg_s)) - starts[seg_s]
        pos = seg_off[seg_s] + within
        gidx_all[c, pos] = sp[order].astype(np.int16)
        dstl_all[c, pos] = (d_all[m][order] & 127).astype(np.float32)

    # calls per (group, range) stream, split at CALL_MAX (32-granular
    # sizes); per call, per 128-chunk: spans of (tile, p0, p1)
    raw = {}                    # (g, r) -> [(r, off, sz, colbase, spans)]
    colbase = 0
    for g in range(NG):
        ts = list(range(g * G, min((g + 1) * G, TPC)))
        for r in range(R):
            bounds = []         # (tile, stream_lo, stream_hi) rel to seg0
            lo = 0
            for t in ts:
                bounds.append((t, lo, lo + int(K[t, r])))
                lo += int(K[t, r])
            total = lo
            base = int(seg_off[ts[0] * R + r])
            cs = 0
            lst = []
            while cs < total:
                sz = min(CALL_MAX, total - cs)
                nch = (sz + 127) // 128
                spans = []
                for k in range(nch):
                    c_lo = cs + 128 * k
                    c_hi = min(cs + 128 * (k + 1), cs + sz)
                    chs = []
                    for (t, s_lo, s_hi) in bounds:
                        a, b = max(c_lo, s_lo), min(c_hi, s_hi)
                        if a >= b:
                            continue
                        p0, p1 = a - c_lo, b - c_lo
                        # PE operand partition windows: base 0 (any len),
                        # base 32 (<=32), base 64 (<=64); base 96 illegal
                        # (excluded by the K bump above)
                        if p0 == 32 and p1 > 64:
                            pieces = [(32, 64), (64, p1)]
                        else:
                            pieces = [(p0, p1)]
                        for (q0, q1) in pieces:
                            chs.append((t, q0, q1))
                    spans.append(chs)
                lst.append((r, base + cs, sz, colbase, spans))
                colbase += nch
                cs += sz
            raw[(g, r)] = lst
    C_total = colbase

    def _finalize(order):
        """Annotate spans with first/last flags for a given emission
        order of (g, r) segments."""
        seq = [c for gr in order for c in raw[gr]]
        first, last = {}, {}
        i = 0
        for (r, off, sz, cb, spans) in seq:
            for chs in spans:
                for (t, p0, p1) in chs:
                    first.setdefault(t, i)
                    last[t] = i
                    i += 1
        out = []
        i = 0
        for (r, off, sz, cb, spans) in seq:
            spans2 = []
            for chs in spans:
                chs2 = []
                for (t, p0, p1) in chs:
                    chs2.append((t, p0, p1, i == first[t], i == last[t]))
                    i += 1
                spans2.append(chs2)
            out.append((r, off, sz, cb, spans2))
        return out

    order_seq = [(g, r) for g in range(NG) for r in range(R)]
    # lookahead-1: queue the next group's r0/r1 gathers before this
    # group's r2 (which waits on the LAST AllGather chunk at ramp time);
    # keeps <= 2 groups of PSUM accumulators live (8 banks -- layer 1 only)
    order_la = [(0, 0), (1, 0), (0, 1), (1, 1), (0, 2)]
    for g in range(2, NG):
        order_la += [(g, 0), (g, 1), (g - 1, 2)]
    order_la.append((NG - 1, 2))
    calls = _finalize(order_seq)
    calls_la = _finalize(order_la)

    idxw_all = np.zeros((N_CORES, 128, T_pad // 16), np.int16)
    for c in range(N_CORES):
        blk = gidx_all[c].reshape(T_pad // 16, 16).T
        idxw_all[c] = np.tile(blk, (8, 1))

    # dstv: column = call colbase + chunk, partition = slot within chunk
    dstv_all = np.full((N_CORES, 128, C_total), -1.0, np.float32)
    for (r, off, sz, cb, spans) in calls:
        nch = (sz + 127) // 128
        for k in range(nch):
            w = min(128, sz - 128 * k)
            dstv_all[:, :w, cb + k] = dstl_all[:, off + 128 * k:
                                               off + 128 * k + w]

    return dict(K=K, calls=calls, calls_la=calls_la, T_pad=T_pad,
                C_total=C_total, idxw=idxw_all, dstv=dstv_all)


def _preprocess(edge_index):
    src = np.asarray(edge_index[0], np.int64)
    dst = np.asarray(edge_index[1], np.int64)
    deg = np.bincount(dst, minlength=N_NODES).astype(np.float64) + 1.0
    dinv = (1.0 / np.sqrt(deg)).astype(np.float32)
    tile_g = dst >> 7

    core_of_tile = np.minimum(np.arange(NT) // TPC, N_CORES - 1)
    tot_tile = np.bincount(tile_g, minlength=NT)
    tile_of = -np.ones((N_CORES, TPC), np.int64)
    for c in range(N_CORES):
        tl = np.where(core_of_tile == c)[0]
        order = tl[np.argsort(-tot_tile[tl], kind='stable')]
        tile_of[c, :len(order)] = order
    rank_of_tile = np.zeros(NT, np.int64)
    for c in range(N_CORES):
        real = tile_of[c][tile_of[c] >= 0]
        rank_of_tile[real] = np.arange(len(real))

    chunk_of = np.repeat(np.arange(R), CH_TILES)    # rank -> chunk

    # Both z tables are RANK-ordered (the host permutes x tiles by
    # tile_of), so both layers share ONE gather metadata set; self-loops
    # are excluded here and handled by a contiguous per-tile DMA +
    # identity matmul inside the aggregation phases.
    t_s = src >> 7
    c_s = core_of_tile[t_s]
    rk_s = rank_of_tile[t_s]
    k_s = chunk_of[rk_s]
    idx = (c_s * np.asarray(RK)[k_s] + (rk_s - np.asarray(CH_START)[k_s])
           * 128 + (src & 127))

    L = _layer_pre(k_s, idx, dst, core_of_tile, tile_of, rank_of_tile)

    dinvS = np.ones((N_CORES, 128, TPC), np.float32)
    for c in range(N_CORES):
        for t in range(TPC):
            tl = tile_of[c, t]
            if tl >= 0:
                dinvS[c, :, t] = dinv[tl * 128:(tl + 1) * 128]

    # packed-input column offsets (f32 pack / i16 pack)
    C = L["C_total"]
    off = {}
    o = 0
    for name, w in [("xT", ROWS), ("w1", HID), ("w2", OUT_CH),
                    ("dinvS", TPC), ("bb1", HID),
                    ("bb2", OUT_CH)]:
        off[name] = o
        o += w
    off["_ftotal"] = o
    off["idx"] = 0
    off["dstv"] = L["T_pad"] // 16
    off["iota"] = L["T_pad"] // 16 + C
    off["_itotal"] = ((off["iota"] + SB * 128 + 1) // 2) * 2

    return dict(dinv=dinv, tile_of=tile_of, L=L, dinvS=dinvS, off=off)


def _emit_agg(nc, tc, src_aps, idx_sb, dstv_sb, dinv_sb, bb_sb, iota8,
              L, layer, out_d, self_src=None, idf=None, w2bf=None,
              bb2_sb=None):
    """Emit one aggregation phase: gather 256B fp16 (hi|lo) rows of
    src_aps[r] per edge and reduce with a one-hot scatter-matmul on the PE
    at fp16 rate -- TWO accumulating matmuls per 128-slot chunk (hi half +
    lo half) recover full f32 precision since lo = z - f32(fp16(z)).

    layer 1 (natural): acc[128dst, 64] = sum S8^T row-select; epilogue is
      z2 = relu(dinv*acc + b1) * dinv -> out_d rows (fp16 hi|lo table).
    layer 2 (swapped): accT[64, 128dst] = msgs^T @ S8; epilogue feeds the
      final out = dinv*(accT^T @ W2) + b2 matmul directly -> out_d rows.
    """
    calls = L["calls_la"]
    tg = f"l{layer}"

    def epilogue(t, psum_t, ep, psO):
        if layer == 1:
            t1 = ep.tile([128, HID], F32, tag=f"t1{tg}")
            nc.vector.tensor_scalar(out=t1[:], in0=psum_t[:],
                                    scalar1=dinv_sb[:, t:t + 1],
                                    scalar2=None,
                                    op0=mybir.AluOpType.mult)
            t2 = ep.tile([128, HID], F32, tag=f"t2{tg}")
            nc.vector.tensor_tensor(out=t2[:], in0=t1[:], in1=bb_sb[:],
                                    op=mybir.AluOpType.add)
            z2 = ep.tile([128, HID], F32, tag=f"z2{tg}")
            nc.scalar.activation(out=z2[:], in_=t2[:],
                                 func=mybir.ActivationFunctionType.Relu,
                                 scale=dinv_sb[:, t:t + 1])
            zcat = ep.tile([128, 2 * HID], FP16, tag=f"zc{tg}")
            nc.scalar.copy(out=zcat[:, 0:HID], in_=z2[:])
            nc.vector.tensor_tensor(out=zcat[:, HID:2 * HID], in0=z2[:],
                                    in1=zcat[:, 0:HID],
                                    op=mybir.AluOpType.subtract)
            nc.scalar.dma_start(out=out_d[t * 128:(t + 1) * 128, :],
                                in_=zcat[:])
        else:
            zT = ep.tile([HID, 128], F32, tag=f"zT{tg}")
            nc.vector.tensor_copy(out=zT[:], in_=psum_t[0:HID, :])
            opt = psO.tile([128, 128], F32, space="PSUM", tag=f"acc{tg}")
            op = opt[:, 0:OUT_CH]
            nc.tensor.matmul(out=op, lhsT=zT[:], rhs=w2bf[:],
                             start=True, stop=True)
            o1 = ep.tile([128, OUT_CH], F32, tag=f"o1{tg}")
            nc.vector.tensor_scalar(out=o1[:], in0=op,
                                    scalar1=dinv_sb[:, t:t + 1],
                                    scalar2=None,
                                    op0=mybir.AluOpType.mult)
            o2 = ep.tile([128, OUT_CH], FP16, tag=f"o2{tg}")
            nc.vector.tensor_tensor(out=o2[:], in0=o1[:], in1=bb2_sb[:],
                                    op=mybir.AluOpType.add)
            nc.scalar.dma_start(out=out_d[t * 128:(t + 1) * 128, :],
                                in_=o2[:])

    from contextlib import ExitStack
    with ExitStack() as stack:
        mp = stack.enter_context(tc.tile_pool(name=f"msgs{layer}",
                                              bufs=MBUFS))
        msp = stack.enter_context(tc.tile_pool(name=f"mself{layer}",
                                               bufs=6))
        sp = stack.enter_context(tc.tile_pool(name=f"s8_{layer}", bufs=6))
        ep = stack.enter_context(tc.tile_pool(name=f"ep{layer}", bufs=8))
        ps = stack.enter_context(
            tc.tile_pool(name=f"ps{layer}", bufs=8, space="PSUM"))
        psO = ps
        qn = 0
        acc = {}
        for (r, off, sz, cb, spans) in calls:
            nch = (sz + 127) // 128
            m = mp.tile([128, CALL_MAX // 128, 2 * HID], FP16,
                        tag=f"msgs{tg}")
            nc.gpsimd.dma_gather(
                out_ap=m[:, :nch, :],
                in_ap=src_aps[r],
                idxs_ap=idx_sb[:, off // 16:(off + sz) // 16],
                num_idxs=sz,
                num_idxs_reg=sz,
                elem_size=2 * HID,
                single_packet=True,
                queue_num=qn % 4,
            )
            qn += 1
            S8 = sp.tile([128, CALL_MAX // 128, 128], FP16, tag=f"s8{tg}")
            dv = dstv_sb[:, cb:cb + nch, None].to_broadcast([128, nch, 128])
            nc.vector.tensor_tensor(
                out=S8[:, :nch, :], in0=iota8[:, :nch, :],
                in1=dv, op=mybir.AluOpType.is_equal)
            for k, chs in enumerate(spans):
                for (t, p0, p1, isf, isl) in chs:
                    if t not in acc:
                        shape = [128, HID] if layer == 1 else [128, 128]
                        acc[t] = ps.tile(shape, F32, space="PSUM",
                                         name=f"acc{tg}_{t}",
                                         tag=f"acc{tg}")
                    if isf:
                        # self-loop term: the tile's own (hi|lo) table rows
                        # via one contiguous DMA + identity matmuls (keeps
                        # 80k rows per layer out of the gather queues)
                        ms = msp.tile([128, 2 * HID], FP16, tag=f"ms{tg}")
                        nc.scalar.dma_start(
                            out=ms[:],
                            in_=self_src[t * 128:(t + 1) * 128, :])
                        if layer == 1:
                            nc.tensor.matmul(out=acc[t][:, :], lhsT=idf[:],
                                             rhs=ms[:, 0:HID],
                                             start=True, stop=False)
                            nc.tensor.matmul(out=acc[t][:, :], lhsT=idf[:],
                                             rhs=ms[:, HID:2 * HID],
                                             start=False, stop=False)
                        else:
                            nc.tensor.matmul(out=acc[t][0:HID, :],
                                             lhsT=ms[:, 0:HID], rhs=idf[:],
                                             start=True, stop=False)
                            nc.tensor.matmul(out=acc[t][0:HID, :],
                                             lhsT=ms[:, HID:2 * HID],
                                             rhs=idf[:],
                                             start=False, stop=False)
                    if layer == 1:
                        nc.tensor.matmul(out=acc[t][:, :],
                                         lhsT=S8[p0:p1, k, :],
                                         rhs=m[p0:p1, k, 0:HID],
                                         start=False, stop=False)
                        nc.tensor.matmul(out=acc[t][:, :],
                                         lhsT=S8[p0:p1, k, :],
                                         rhs=m[p0:p1, k, HID:2 * HID],
                                         start=False, stop=isl)
                    else:
                        nc.tensor.matmul(out=acc[t][0:HID, :],
                                         lhsT=m[p0:p1, k, 0:HID],
                                         rhs=S8[p0:p1, k, :],
                                         start=False, stop=False)
                        nc.tensor.matmul(out=acc[t][0:HID, :],
                                         lhsT=m[p0:p1, k, HID:2 * HID],
                                         rhs=S8[p0:p1, k, :],
                                         start=False, stop=isl)
                    if isl:
                        epilogue(t, acc.pop(t), ep, psO)


def _build_merged(pre, prefix=5):
    L, off = pre["L"], pre["off"]
    nc = bacc.Bacc("TRN2", target_bir_lowering=False, debug=False,
                   num_devices=N_CORES, num_swdge_queues=4,
                   dynamic_dma_scratch_size=16384)
    pf_d = nc.dram_tensor("pf", [128, off["_ftotal"] + off["_itotal"] // 2],
                          F32, kind="ExternalInput")
    out_d = nc.dram_tensor("outp", [ROWS, OUT_CH], FP16,
                           kind="ExternalOutput")


    with tile.TileContext(nc) as tc:
        with (
            tc.tile_pool(name="const", bufs=1) as cp,
            tc.tile_pool(name="dram", bufs=1, space="DRAM") as dram,
        ):
            ag1_in = dram.tile([ROWS, 2 * HID], FP16)
            ag2_in = dram.tile([ROWS, 2 * HID], FP16)
            _aspace = "Local" if globals().get("_NO_CC", False) else "Shared"
            ag1_out = [dram.tile([N_CORES * RK[k], 2 * HID], FP16,
                                 addr_space=_aspace, name=f"ag1o{k}")
                       for k in range(R)]
            ag2_out = [dram.tile([N_CORES * RK[k], 2 * HID], FP16,
                                 addr_space=_aspace, name=f"ag2o{k}")
                       for k in range(R)]

            def pf_load(name, cols, parts=128):
                t = cp.tile([parts, cols], F32, name=f"c_{name}",
                            tag=f"c_{name}")
                nc.sync.dma_start(
                    out=t[:],
                    in_=pf_d.ap()[:parts, off[name]:off[name] + cols])
                return t

            w1sb = pf_load("w1", HID)
            w2f = pf_load("w2", OUT_CH, parts=HID)
            w2bf = w2f
            dinvS_sb = pf_load("dinvS", TPC)
            bb1_sb = pf_load("bb1", HID)
            bb2_sb = pf_load("bb2", OUT_CH)
            pi_ap16 = pf_d.ap()[:, off["_ftotal"]:].bitcast(I16)
            dstv_sb = cp.tile([128, L["C_total"]], FP16, name="c_dstv",
                              tag="c_dstv")
            nc.sync.dma_start(
                out=dstv_sb[:],
                in_=pi_ap16[:, off["dstv"]:off["dstv"] + L["C_total"]]
                .bitcast(FP16))
            idx_sb = cp.tile([128, L["T_pad"] // 16], I16)
            nc.sync.dma_start(
                out=idx_sb[:],
                in_=pi_ap16[:, off["idx"]:off["idx"] + L["T_pad"] // 16])
            identf = cp.tile([128, 128], F32)
            make_identity(nc, identf[:])
            idf = cp.tile([128, 128], FP16)
            nc.vector.tensor_copy(out=idf[:], in_=identf[:])
            iota8 = cp.tile([128, SB, 128], FP16)
            nc.scalar.dma_start(
                out=iota8[:],
                in_=pi_ap16[:, off["iota"]:off["iota"] + SB * 128]
                .bitcast(FP16).rearrange("p (c f) -> p c f", c=SB))

            # ---- phase A: z1 = dinvA * (x @ W1) -> ag1_in
            # x arrives pre-transposed from the host: lhsT slices directly.
            with (
                tc.tile_pool(name="xt", bufs=1) as xtp,
                tc.tile_pool(name="zs", bufs=8) as zp,
                tc.tile_pool(name="psA", bufs=8, space="PSUM") as psA,
            ):
                xT_sb = xtp.tile([IN_CH, ROWS], F32)
                for k0 in range(0, TPC, 10):
                    k1 = min(k0 + 10, TPC)
                    nc.sync.dma_start(
                        out=xT_sb[:, k0 * 128:k1 * 128],
                        in_=pf_d.ap()[:, off["xT"] + k0 * 128:
                                      off["xT"] + k1 * 128])
                for t in range(TPC):
                    zps = psA.tile([128, HID], F32, space="PSUM")
                    nc.tensor.matmul(out=zps[:],
                                     lhsT=xT_sb[:, t * 128:(t + 1) * 128],
                                     rhs=w1sb[:], start=True, stop=True)
                    zsb = zp.tile([128, HID], F32, tag="zsb")
                    nc.vector.tensor_scalar(out=zsb[:], in0=zps[:],
                                            scalar1=dinvS_sb[:, t:t + 1],
                                            scalar2=None,
                                            op0=mybir.AluOpType.mult)
                    zcat = zp.tile([128, 2 * HID], FP16, tag="zcat")
                    nc.vector.tensor_copy(out=zcat[:, 0:HID], in_=zsb[:])
                    nc.vector.tensor_tensor(out=zcat[:, HID:2 * HID],
                                            in0=zsb[:],
                                            in1=zcat[:, 0:HID],
                                            op=mybir.AluOpType.subtract)
                    nc.scalar.dma_start(out=ag1_in[t * 128:(t + 1) * 128, :],
                                        in_=zcat[:])

            no_cc = globals().get("_NO_CC", False)
            if prefix >= 2:
                for k in range(R):
                    lo = 128 * CH_START[k]
                    if no_cc:
                        for cc in range(N_CORES):
                            nc.sync.dma_start(
                                out=ag1_out[k][cc * RK[k]:(cc + 1) * RK[k], :],
                                in_=ag1_in[lo:lo + RK[k], :])
                    else:
                        nc.gpsimd.collective_compute(
                            "AllGather", mybir.AluOpType.bypass,
                            replica_groups=[list(range(N_CORES))],
                            ins=[ag1_in[lo:lo + RK[k], :]],
                            outs=[ag1_out[k][:]])

            if prefix >= 3:
                _emit_agg(nc, tc, [a[:] for a in ag1_out], idx_sb, dstv_sb,
                          dinvS_sb, bb1_sb, iota8, L, layer=1,
                          out_d=ag2_in, self_src=ag1_in, idf=idf)

            if prefix >= 4:
                for k in range(R):
                    lo = 128 * CH_START[k]
                    if no_cc:
                        for cc in range(N_CORES):
                            nc.sync.dma_start(
                                out=ag2_out[k][cc * RK[k]:(cc + 1) * RK[k], :],
                                in_=ag2_in[lo:lo + RK[k], :])
                    else:
                        nc.gpsimd.collective_compute(
                            "AllGather", mybir.AluOpType.bypass,
                            replica_groups=[list(range(N_CORES))],
                            ins=[ag2_in[lo:lo + RK[k], :]],
                            outs=[ag2_out[k][:]])

            if prefix >= 5:
                _emit_agg(nc, tc, [a[:] for a in ag2_out], idx_sb, dstv_sb,
                          dinvS_sb, bb1_sb, iota8, L, layer=2,
                          out_d=out_d.ap(), self_src=ag2_in, idf=idf,
                          w2bf=w2bf, bb2_sb=bb2_sb)
    nc.compile()
    return nc


class _SpmdRunner:
    def __init__(self, nc, n_cores=N_CORES):
        bass2jax.install_neuronx_cc_hook()
        self.nc = nc
        self.n_cores = n_cores
        in_names, out_names, out_avals = [], [], []
        partition_name = nc.partition_id_tensor.name if nc.partition_id_tensor \
            else None
        for alloc in nc.m.functions[0].allocations:
            if not isinstance(alloc, mybir.MemoryLocationSet):
                continue
            name = alloc.memorylocations[0].name
            if alloc.kind == "ExternalInput":
                if name != partition_name:
                    in_names.append(name)
            elif alloc.kind == "ExternalOutput":
                out_names.append(name)
                out_avals.append(jax.core.ShapedArray(
                    tuple(alloc.tensor_shape), mybir.dt.np(alloc.dtype)))
        self.in_names, self.out_names, self.out_avals = \
            in_names, out_names, out_avals
        n_params = len(in_names)
        n_outs = len(out_avals)
        all_names = list(in_names) + list(out_names)
        if partition_name is not None:
            all_names.append(partition_name)

        def _body(*args):
            operands = list(args)
            if partition_name is not None:
                operands.append(partition_id_tensor())
            outs = _bass_exec_p.bind(
                *operands,
                out_avals=tuple(out_avals),
                in_names=tuple(all_names),
                out_names=tuple(out_names),
                lowering_input_output_aliases=(),
                sim_require_finite=True,
                sim_require_nnan=True,
                nc=nc,
            )
            return tuple(outs)

        devices = jax.devices()[:n_cores]
        assert len(devices) >= n_cores or len(devices) == n_cores, \
            f"need {n_cores} cores, have {len(jax.devices())}"
        self.mesh = Mesh(np.asarray(devices), ("core",))
        in_specs = (PartitionSpec("core"),) * (n_params + n_outs)
        out_specs = (PartitionSpec("core"),) * n_outs
        self.fn = jax.jit(
            shard_map(_body, mesh=self.mesh, in_specs=in_specs,
                      out_specs=out_specs, check_rep=False),
            keep_unused=True,
        )

    def run(self, in_maps):
        concat_in = [
            np.concatenate([np.asarray(in_maps[c][nm])
                            for c in range(self.n_cores)], axis=0)
            for nm in self.in_names
        ]
        concat_zeros = [
            np.zeros((self.n_cores * av.shape[0], *av.shape[1:]), av.dtype)
            for av in self.out_avals
        ]
        outs = self.fn(*(concat_in + concat_zeros))
        jax.block_until_ready(outs)
        res = []
        for c in range(self.n_cores):
            d = {}
            for i, nm in enumerate(self.out_names):
                a = np.asarray(outs[i]).reshape(self.n_cores,
                                                *self.out_avals[i].shape)
                d[nm] = a[c]
            res.append(d)
        return res


_CACHE = {}


def _get_programs(edge_index):
    key = hash(np.asarray(edge_index).tobytes())
    if key not in _CACHE:
        pre = _preprocess(edge_index)
        ncM = _build_merged(pre)
        _CACHE[key] = (pre, _SpmdRunner(ncM))
    return _CACHE[key]


def _make_maps(pre, x, W1, b1, W2, b2):
    off = pre["off"]
    tile_of = pre["tile_of"]
    bb1 = np.tile(b1, (128, 1)).astype(np.float32)
    bb2 = np.tile(b2, (128, 1)).astype(np.float32)
    w2pad = np.zeros((128, OUT_CH), np.float32)
    w2pad[:HID] = W2
    maps = []
    for c in range(N_CORES):
        xs = np.zeros((ROWS, IN_CH), np.float32)
        for t in range(TPC):
            tl = tile_of[c, t]
            if tl >= 0:
                xs[t * 128:(t + 1) * 128] = x[tl * 128:(tl + 1) * 128]
        pf = np.empty((128, off["_ftotal"]), np.float32)
        pf[:, off["xT"]:off["xT"] + ROWS] = xs.T
        pf[:, off["w1"]:off["w1"] + HID] = W1
        pf[:, off["w2"]:off["w2"] + OUT_CH] = w2pad
        pf[:, off["dinvS"]:off["dinvS"] + TPC] = pre["dinvS"][c]
        pf[:, off["bb1"]:off["bb1"] + HID] = bb1
        pf[:, off["bb2"]:off["bb2"] + OUT_CH] = bb2
        pi = np.zeros((128, off["_itotal"]), np.int16)
        pi[:, :off["dstv"]] = pre["L"]["idxw"][c]
        pi[:, off["dstv"]:off["dstv"] + pre["L"]["C_total"]] = \
            pre["L"]["dstv"][c].astype(np.float16).view(np.int16)
        iota = np.tile(np.arange(128, dtype=np.float16), SB)
        pi[:, off["iota"]:off["iota"] + SB * 128] = \
            np.tile(iota.view(np.int16), (128, 1))
        maps.append({"pf": np.concatenate([pf, pi.view(np.float32)],
                                          axis=1)})
    return maps


def kernel(x, edge_index, W1, b1, W2, b2):
    x = np.asarray(x, np.float32)
    W1 = np.asarray(W1, np.float32)
    b1 = np.asarray(b1, np.float32)
    W2 = np.asarray(W2, np.float32)
    b2 = np.asarray(b2, np.float32)
    pre, rM = _get_programs(edge_index)
    maps = _make_maps(pre, x, W1, b1, W2, b2)
    res = rM.run(maps)
    tile_of = pre["tile_of"]
    out = np.zeros((N_NODES, OUT_CH), np.float32)
    for c in range(N_CORES):
        o = np.asarray(res[c]["outp"], np.float32)
        for t in range(TPC):
            tl = tile_of[c, t]
            if tl >= 0:
                out[tl * 128:(tl + 1) * 128] = o[t * 128:(t + 1) * 128]
    return out


# revision 39
# speedup vs baseline: 1.0133x; 1.0133x over previous
"""GCN 2-layer encoder on 8 Trainium2 NeuronCores (Bass/Tile).

kernel(**inputs) takes the FULL inputs and returns the FULL [80000, 32] f32
output.  Strategy (node partition across 8 cores, per sharding hint), ONE
fused SPMD launch with chunked in-kernel AllGathers:

  gcn_conv(x, W, b) = b + dinv * (A_hat @ (dinv * (x @ W)))  with self-loops,
  where dinv = 1/sqrt(indeg+1) and A_hat is the (unnormalized) adjacency.

  Both z tables are RANK-ordered (the host permutes x tiles by tile_of), so
  the two layers share ONE gather metadata set, and each dst tile's
  self-loop rows sit at a per-rank-uniform LOCAL offset (SPMD-identical
  across cores) -- they are lifted out of the gather into one contiguous
  DMA + identity matmuls per tile.

  z tables are stored as fp16 (hi | lo) pairs in 256B rows, lo = z - f32(hi):
  dma_gather moves 256B per row regardless (descriptor-bound, ~2.4ns/row on
  4 SWDGE queues), so the lo half rides for free and TWO accumulating fp16
  matmuls per 128-slot chunk recover full f32 precision at the fp16 PE rate
  (4x the f32 rate).

  Phase A: z1 = dinvS * (x @ W1) -> hi|lo -> ag1_in   (rank-ordered shard)
  AllGather(z1) in 3 chunks of 27/27/25 tile-ranks; chunk k fires as soon
           as phase A finishes those ranks; each chunk is one int16 gather
           range (<= 27648 rows).
  Phase B: per dst tile, gather 256B rows by edge source (gpsimd dma_gather,
           calls of <=1024 rows grouped over 4 dst tiles), reduce via
           one-hot scatter-matmul (lhsT=S8 fp16, rhs=msgs hi/lo) into PSUM;
           self rows via direct DMA + identity matmul; epilogue
           z2 = relu(dinv*agg + b1) * dinv -> hi|lo -> ag2_in
           (relu on the otherwise-idle ACT engine, hi/lo split on DVE+ACT).
  AllGather(z2) in 3 chunks (same overlap).
  Phase C: same gathers over z2 but the scatter-matmul runs with SWAPPED
           operands (lhsT=messages, rhs=one-hot) accumulating the
           TRANSPOSED aggregate accT[64,128], which feeds the final
           out = dinv*(accT^T @ W2) + b2 matmul directly per tile (no PE
           transpose pass, no separate final phase).  Output fp16 (host
           casts back; tolerance 2e-2 >> fp16 eps).

  Edge slots are grouped by (dst-tile group, src chunk, dst tile) with
  counts padded to a 32-multiple of the global per-rank max so all 8 cores
  run one identical SPMD program (PE operand partition windows allow bases
  0/32/64, so a 128-slot chunk straddling two tiles is reduced by
  partition-sliced matmul spans); gather indices sorted by source address
  for HBM locality.

  ALL inputs are packed into ONE f32 dram tensor (idx/dstv/iota ride as
  int16/fp16 bitcast regions): the per-launch dispatch floor scales with
  arg count (~56us/arg), so 13 args -> 2 is worth ~600us/launch.  Static
  DMAs are spread across both HWDGE queues (SP + ACT engines).
"""
import sys

sys.path.insert(0, '/opt/trn_rl_repo')

import numpy as np
import jax
from jax.sharding import Mesh, PartitionSpec
from jax.experimental.shard_map import shard_map

import concourse.bass as bass
import concourse.bacc as bacc
import concourse.tile as tile
import concourse.mybir as mybir
from concourse import bass2jax
from concourse.bass2jax import _bass_exec_p, partition_id_tensor
from concourse.masks import make_identity

F32 = mybir.dt.float32
BF16 = mybir.dt.bfloat16
FP16 = mybir.dt.float16
I16 = mybir.dt.int16

N_NODES = 80000
IN_CH = 128
HID = 64
OUT_CH = 32
N_CORES = 8
NT = N_NODES // 128                     # 625 dst tiles
TPC = (NT + N_CORES - 1) // N_CORES     # 79 tile ranks per core
ROWS = TPC * 128                        # 10112 rows per core
CH_TILES = [27, 27, 25]                 # AllGather chunking (tiles per chunk)
CH_START = [0, 27, 54]
RK = [t * 128 for t in CH_TILES]        # rows per core per chunk
R = 3                                   # one gather range per chunk
G = 4                                   # dst tiles per gather-call group
NG = (TPC + G - 1) // G
CALL_MAX = 1024                         # max indices per dma_gather call
SB = 16                                 # one-hot S matrices built per batch
MBUFS = 20                              # msgs pool double-buffer depth


def _ceil32(x):
    return ((x + 31) // 32) * 32


def _layer_pre(r_id, ridx, d_all, core_of_tile, tile_of, rank_of_tile):
    """Gather/scatter metadata (shared by both layers).

    Slot stream: segments (tile, range) in (group, range, tile) order,
    each padded only to a 32 multiple of the max-over-cores edge count
    (PE matmul partition bases must be 0/32/64/96).  Calls split the
    stream at <=CALL_MAX; a call-relative 128-chunk may straddle two
    segments -> per-chunk SPANS (tile, p0, p1, first, last) drive
    partition-sliced matmuls.
    """
    tile_g = d_all >> 7
    cnt_tile_r = np.bincount(tile_g * R + r_id,
                             minlength=NT * R).reshape(NT, R)
    cntK = np.zeros((N_CORES, TPC, R), np.int64)
    for c in range(N_CORES):
        real = tile_of[c][tile_of[c] >= 0]
        cntK[c, :len(real)] = cnt_tile_r[real]
    K = _ceil32(cntK.max(axis=0))       # [TPC, R] padded slot counts
    # PE matmul partition bases must be 0/32/64 (96 is rejected): bump any
    # segment whose cumulative in-stream boundary would land at 96 mod 128
    for g in range(NG):
        ts = list(range(g * G, min((g + 1) * G, TPC)))
        for r in range(R):
            cum = 0
            for t in ts:
                cum += int(K[t, r])
                if cum % 128 == 96:
                    K[t, r] += 32
                    cum += 32

    # segment order: (group, range, tile-in-group)
    seg_order = []
    for g in range(NG):
        ts = list(range(g * G, min((g + 1) * G, TPC)))
        for r in range(R):
            for t in ts:
                seg_order.append(t * R + r)
    seg_order = np.asarray(seg_order)
    seg_pos = np.empty(TPC * R, np.int64)
    seg_pos[seg_order] = np.arange(TPC * R)
    Kflat = K.reshape(-1)
    Koff_ord = np.zeros(TPC * R + 1, np.int64)
    Koff_ord[1:] = np.cumsum(Kflat[seg_order])
    T_pad = int(Koff_ord[-1])
    seg_off = np.empty(TPC * R, np.int64)
    seg_off[seg_order] = Koff_ord[:-1]

    # pad slots gather *scattered* rows (same-row reads serialize on one
    # HBM bank); dstl=-1 keeps them out of the one-hot reduction
    rngpad = np.random.default_rng(12345)
    sizes = np.asarray([N_CORES * r for r in RK])
    gidx_all = np.empty((N_CORES, T_pad), np.int16)
    for r in range(R):
        for g in range(NG):
            ts = list(range(g * G, min((g + 1) * G, TPC)))
            lo = int(seg_off[ts[0] * R + r])
            hi = lo + int(sum(K[t, r] for t in ts))
            gidx_all[:, lo:hi] = rngpad.integers(
                0, sizes[r], (N_CORES, hi - lo)).astype(np.int16)
    dstl_all = np.full((N_CORES, T_pad), -1.0, np.float32)
    ecore = core_of_tile[tile_g]
    for c in range(N_CORES):
        m = ecore == c
        e_rank = rank_of_tile[tile_g[m]]
        e_r = r_id[m]
        seg = e_rank * R + e_r
        sp = ridx[m]
        order = np.lexsort((sp, seg_pos[seg]))
        seg_s = seg[order]                      # sorted by segment ordinal
        seg_counts = np.bincount(seg_s, minlength=TPC * R)
        starts = np.zeros(TPC * R, np.int64)    # seg id -> stream start
        csum = np.cumsum(seg_counts[seg_order])
        starts[seg_order[1:]] = csum[:-1]
        within = np.arange(len(seg_s)) - starts[seg_s]
        pos = seg_off[seg_s] + within
        gidx_all[c, pos] = sp[order].astype(np.int16)
        dstl_all[c, pos] = (d_all[m][order] & 127).astype(np.float32)

    # calls per (group, range) stream, split at CALL_MAX (32-granular
    # sizes); per call, per 128-chunk: spans of (tile, p0, p1)
    raw = {}                    # (g, r) -> [(r, off, sz, colbase, spans)]
    colbase = 0
    for g in range(NG):
        ts = list(range(g * G, min((g + 1) * G, TPC)))
        for r in range(R):
            bounds = []         # (tile, stream_lo, stream_hi) rel to seg0
            lo = 0
            for t in ts:
                bounds.append((t, lo, lo + int(K[t, r])))
                lo += int(K[t, r])
            total = lo
            base = int(seg_off[ts[0] * R + r])
            cs = 0
            lst = []
            while cs < total:
                sz = min(CALL_MAX, total - cs)
                nch = (sz + 127) // 128
                spans = []
                for k in range(nch):
                    c_lo = cs + 128 * k
                    c_hi = min(cs + 128 * (k + 1), cs + sz)
                    chs = []
                    for (t, s_lo, s_hi) in bounds:
                        a, b = max(c_lo, s_lo), min(c_hi, s_hi)
                        if a >= b:
                            continue
                        p0, p1 = a - c_lo, b - c_lo
                        # PE operand partition windows: base 0 (any len),
                        # base 32 (<=32), base 64 (<=64); base 96 illegal
                        # (excluded by the K bump above)
                        if p0 == 32 and p1 > 64:
                            pieces = [(32, 64), (64, p1)]
                        else:
                            pieces = [(p0, p1)]
                        for (q0, q1) in pieces:
                            chs.append((t, q0, q1))
                    spans.append(chs)
                lst.append((r, base + cs, sz, colbase, spans))
                colbase += nch
                cs += sz
            raw[(g, r)] = lst
    C_total = colbase

    def _finalize(order):
        """Annotate spans with first/last flags for a given emission
        order of (g, r) segments."""
        seq = [c for gr in order for c in raw[gr]]
        first, last = {}, {}
        i = 0
        for (r, off, sz, cb, spans) in seq:
            for chs in spans:
                for (t, p0, p1) in chs:
                    first.setdefault(t, i)
                    last[t] = i
                    i += 1
        out = []
        i = 0
        for (r, off, sz, cb, spans) in seq:
            spans2 = []
            for chs in spans:
                chs2 = []
                for (t, p0, p1) in chs:
                    chs2.append((t, p0, p1, i == first[t], i == last[t]))
                    i += 1
                spans2.append(chs2)
            out.append((r, off, sz, cb, spans2))
        return out

    order_seq = [(g, r) for g in range(NG) for r in range(R)]
    # lookahead-1: queue the next group's r0/r1 gathers before this
    # group's r2 (which waits on the LAST AllGather chunk at ramp time);
    # keeps <= 2 groups of PSUM accumulators live (8 banks -- layer 1 only)
    order_la = [(0, 0), (1, 0), (0, 1), (1, 1), (0, 2)]
    for g in range(2, NG):
        order_la += [(g, 0), (g, 1), (g - 1, 2)]
    order_la.append((NG - 1, 2))
    calls = _finalize(order_seq)
    calls_la = _finalize(order_la)

    idxw_all = np.zeros((N_CORES, 128, T_pad // 16), np.int16)
    for c in range(N_CORES):
        blk = gidx_all[c].reshape(T_pad // 16, 16).T
        idxw_all[c] = np.tile(blk, (8, 1))

    # dstv: column = call colbase + chunk, partition = slot within chunk
    dstv_all = np.full((N_CORES, 128, C_total), -1.0, np.float32)
    for (r, off, sz, cb, spans) in calls:
        nch = (sz + 127) // 128
        for k in range(nch):
            w = min(128, sz - 128 * k)
            dstv_all[:, :w, cb + k] = dstl_all[:, off + 128 * k:
                                               off + 128 * k + w]

    return dict(K=K, calls=calls, calls_la=calls_la, T_pad=T_pad,
                C_total=C_total, idxw=idxw_all, dstv=dstv_all)


def _preprocess(edge_index):
    src = np.asarray(edge_index[0], np.int64)
    dst = np.asarray(edge_index[1], np.int64)
    deg = np.bincount(dst, minlength=N_NODES).astype(np.float64) + 1.0
    dinv = (1.0 / np.sqrt(deg)).astype(np.float32)
    tile_g = dst >> 7

    core_of_tile = np.minimum(np.arange(NT) // TPC, N_CORES - 1)
    tot_tile = np.bincount(tile_g, minlength=NT)
    tile_of = -np.ones((N_CORES, TPC), np.int64)
    for c in range(N_CORES):
        tl = np.where(core_of_tile == c)[0]
        order = tl[np.argsort(-tot_tile[tl], kind='stable')]
        tile_of[c, :len(order)] = order
    rank_of_tile = np.zeros(NT, np.int64)
    for c in range(N_CORES):
        real = tile_of[c][tile_of[c] >= 0]
        rank_of_tile[real] = np.arange(len(real))

    chunk_of = np.repeat(np.arange(R), CH_TILES)    # rank -> chunk

    # Both z tables are RANK-ordered (the host permutes x tiles by
    # tile_of), so both layers share ONE gather metadata set; self-loops
    # are excluded here and handled by a contiguous per-tile DMA +
    # identity matmul inside the aggregation phases.
    t_s = src >> 7
    c_s = core_of_tile[t_s]
    rk_s = rank_of_tile[t_s]
    k_s = chunk_of[rk_s]
    idx = (c_s * np.asarray(RK)[k_s] + (rk_s - np.asarray(CH_START)[k_s])
           * 128 + (src & 127))

    L = _layer_pre(k_s, idx, dst, core_of_tile, tile_of, rank_of_tile)

    dinvS = np.ones((N_CORES, 128, TPC), np.float32)
    for c in range(N_CORES):
        for t in range(TPC):
            tl = tile_of[c, t]
            if tl >= 0:
                dinvS[c, :, t] = dinv[tl * 128:(tl + 1) * 128]

    # packed-input column offsets (f32 pack / i16 pack)
    C = L["C_total"]
    off = {}
    o = 0
    for name, w in [("xT", ROWS), ("w1", HID), ("w2", OUT_CH),
                    ("dinvS", TPC), ("bb1", HID),
                    ("bb2", OUT_CH)]:
        off[name] = o
        o += w
    off["_ftotal"] = o
    off["idx"] = 0
    off["dstv"] = L["T_pad"] // 16
    off["iota"] = L["T_pad"] // 16 + C
    off["_itotal"] = ((off["iota"] + SB * 128 + 1) // 2) * 2

    return dict(dinv=dinv, tile_of=tile_of, L=L, dinvS=dinvS, off=off)


def _emit_agg(nc, tc, ps, src_aps, idx_sb, dstv_sb, dinv_sb, bb_sb, iota8,
              L, layer, out_d, self_src=None, idf=None, w2bf=None,
              bb2_sb=None):
    """Emit one aggregation phase: gather 256B fp16 (hi|lo) rows of
    src_aps[r] per edge and reduce with a one-hot scatter-matmul on the PE
    at fp16 rate -- TWO accumulating matmuls per 128-slot chunk (hi half +
    lo half) recover full f32 precision since lo = z - f32(fp16(z)).

    layer 1 (natural): acc[128dst, 64] = sum S8^T row-select; epilogue is
      z2 = relu(dinv*acc + b1) * dinv -> out_d rows (fp16 hi|lo table).
    layer 2 (swapped): accT[64, 128dst] = msgs^T @ S8; epilogue feeds the
      final out = dinv*(accT^T @ W2) + b2 matmul directly -> out_d rows.
    """
    calls = L["calls_la"]
    tg = f"l{layer}"

    def epilogue(t, psum_t, ep, psO):
        if layer == 1:
            t1 = ep.tile([128, HID], F32, tag=f"t1{tg}")
            nc.vector.tensor_scalar(out=t1[:], in0=psum_t[:, 0:HID],
                                    scalar1=dinv_sb[:, t:t + 1],
                                    scalar2=None,
                                    op0=mybir.AluOpType.mult)
            t2 = ep.tile([128, HID], F32, tag=f"t2{tg}")
            nc.vector.tensor_tensor(out=t2[:], in0=t1[:], in1=bb_sb[:],
                                    op=mybir.AluOpType.add)
            z2 = ep.tile([128, HID], F32, tag=f"z2{tg}")
            nc.scalar.activation(out=z2[:], in_=t2[:],
                                 func=mybir.ActivationFunctionType.Relu,
                                 scale=dinv_sb[:, t:t + 1])
            zcat = ep.tile([128, 2 * HID], FP16, tag=f"zc{tg}")
            nc.scalar.copy(out=zcat[:, 0:HID], in_=z2[:])
            nc.vector.tensor_tensor(out=zcat[:, HID:2 * HID], in0=z2[:],
                                    in1=zcat[:, 0:HID],
                                    op=mybir.AluOpType.subtract)
            nc.scalar.dma_start(out=out_d[t * 128:(t + 1) * 128, :],
                                in_=zcat[:])
        else:
            zT = ep.tile([HID, 128], F32, tag=f"zT{tg}")
            nc.vector.tensor_copy(out=zT[:], in_=psum_t[0:HID, :])
            opt = psO.tile([128, 128], F32, space="PSUM", tag="accS")
            op = opt[:, 0:OUT_CH]
            nc.tensor.matmul(out=op, lhsT=zT[:], rhs=w2bf[:],
                             start=True, stop=True)
            o1 = ep.tile([128, OUT_CH], F32, tag=f"o1{tg}")
            nc.vector.tensor_scalar(out=o1[:], in0=op,
                                    scalar1=dinv_sb[:, t:t + 1],
                                    scalar2=None,
                                    op0=mybir.AluOpType.mult)
            o2 = ep.tile([128, OUT_CH], FP16, tag=f"o2{tg}")
            nc.vector.tensor_tensor(out=o2[:], in0=o1[:], in1=bb2_sb[:],
                                    op=mybir.AluOpType.add)
            nc.scalar.dma_start(out=out_d[t * 128:(t + 1) * 128, :],
                                in_=o2[:])

    from contextlib import ExitStack
    with ExitStack() as stack:
        mp = stack.enter_context(tc.tile_pool(name=f"msgs{layer}",
                                              bufs=MBUFS))
        msp = stack.enter_context(tc.tile_pool(name=f"mself{layer}",
                                               bufs=6))
        sp = stack.enter_context(tc.tile_pool(name=f"s8_{layer}", bufs=6))
        ep = stack.enter_context(tc.tile_pool(name=f"ep{layer}", bufs=8))
        psO = ps
        qn = 0
        acc = {}
        for (r, off, sz, cb, spans) in calls:
            nch = (sz + 127) // 128
            m = mp.tile([128, CALL_MAX // 128, 2 * HID], FP16,
                        tag=f"msgs{tg}")
            nc.gpsimd.dma_gather(
                out_ap=m[:, :nch, :],
                in_ap=src_aps[r],
                idxs_ap=idx_sb[:, off // 16:(off + sz) // 16],
                num_idxs=sz,
                num_idxs_reg=sz,
                elem_size=2 * HID,
                single_packet=True,
                queue_num=qn % 4,
            )
            qn += 1
            S8 = sp.tile([128, CALL_MAX // 128, 128], FP16, tag=f"s8{tg}")
            dv = dstv_sb[:, cb:cb + nch, None].to_broadcast([128, nch, 128])
            nc.vector.tensor_tensor(
                out=S8[:, :nch, :], in0=iota8[:, :nch, :],
                in1=dv, op=mybir.AluOpType.is_equal)
            for k, chs in enumerate(spans):
                for (t, p0, p1, isf, isl) in chs:
                    if t not in acc:
                        acc[t] = ps.tile([128, 128], F32, space="PSUM",
                                         name=f"acc{tg}_{t}", tag="accS")
                    if isf:
                        # self-loop term: the tile's own (hi|lo) table rows
                        # via one contiguous DMA + identity matmuls (keeps
                        # 80k rows per layer out of the gather queues)
                        ms = msp.tile([128, 2 * HID], FP16, tag=f"ms{tg}")
                        nc.scalar.dma_start(
                            out=ms[:],
                            in_=self_src[t * 128:(t + 1) * 128, :])
                        if layer == 1:
                            nc.tensor.matmul(out=acc[t][:, 0:HID], lhsT=idf[:],
                                             rhs=ms[:, 0:HID],
                                             start=True, stop=False)
                            nc.tensor.matmul(out=acc[t][:, 0:HID], lhsT=idf[:],
                                             rhs=ms[:, HID:2 * HID],
                                             start=False, stop=False)
                        else:
                            nc.tensor.matmul(out=acc[t][0:HID, :],
                                             lhsT=ms[:, 0:HID], rhs=idf[:],
                                             start=True, stop=False)
                            nc.tensor.matmul(out=acc[t][0:HID, :],
                                             lhsT=ms[:, HID:2 * HID],
                                             rhs=idf[:],
                                             start=False, stop=False)
                    if layer == 1:
                        nc.tensor.matmul(out=acc[t][:, 0:HID],
                                         lhsT=S8[p0:p1, k, :],
                                         rhs=m[p0:p1, k, 0:HID],
                                         start=False, stop=False)
                        nc.tensor.matmul(out=acc[t][:, 0:HID],
                                         lhsT=S8[p0:p1, k, :],
                                         rhs=m[p0:p1, k, HID:2 * HID],
                                         start=False, stop=isl)
                    else:
                        nc.tensor.matmul(out=acc[t][0:HID, :],
                                         lhsT=m[p0:p1, k, 0:HID],
                                         rhs=S8[p0:p1, k, :],
                                         start=False, stop=False)
                        nc.tensor.matmul(out=acc[t][0:HID, :],
                                         lhsT=m[p0:p1, k, HID:2 * HID],
                                         rhs=S8[p0:p1, k, :],
                                         start=False, stop=isl)
                    if isl:
                        epilogue(t, acc.pop(t), ep, psO)


def _build_merged(pre, prefix=5):
    L, off = pre["L"], pre["off"]
    nc = bacc.Bacc("TRN2", target_bir_lowering=False, debug=False,
                   num_devices=N_CORES, num_swdge_queues=4,
                   dynamic_dma_scratch_size=16384)
    pf_d = nc.dram_tensor("pf", [128, off["_ftotal"] + off["_itotal"] // 2],
                          F32, kind="ExternalInput")
    out_d = nc.dram_tensor("outp", [ROWS, OUT_CH], FP16,
                           kind="ExternalOutput")


    with tile.TileContext(nc) as tc:
        with (
            tc.tile_pool(name="const", bufs=1) as cp,
            tc.tile_pool(name="psShared", bufs=8, space="PSUM") as psS,
            tc.tile_pool(name="dram", bufs=1, space="DRAM") as dram,
        ):
            ag1_in = dram.tile([ROWS, 2 * HID], FP16)
            ag2_in = dram.tile([ROWS, 2 * HID], FP16)
            _aspace = "Local" if globals().get("_NO_CC", False) else "Shared"
            ag1_out = [dram.tile([N_CORES * RK[k], 2 * HID], FP16,
                                 addr_space=_aspace, name=f"ag1o{k}")
                       for k in range(R)]
            ag2_out = [dram.tile([N_CORES * RK[k], 2 * HID], FP16,
                                 addr_space=_aspace, name=f"ag2o{k}")
                       for k in range(R)]

            def pf_load(name, cols, parts=128):
                t = cp.tile([parts, cols], F32, name=f"c_{name}",
                            tag=f"c_{name}")
                nc.sync.dma_start(
                    out=t[:],
                    in_=pf_d.ap()[:parts, off[name]:off[name] + cols])
                return t

            w1sb = pf_load("w1", HID)
            w2f = pf_load("w2", OUT_CH, parts=HID)
            w2bf = w2f
            dinvS_sb = pf_load("dinvS", TPC)
            bb1_sb = pf_load("bb1", HID)
            bb2_sb = pf_load("bb2", OUT_CH)
            pi_ap16 = pf_d.ap()[:, off["_ftotal"]:].bitcast(I16)
            dstv_sb = cp.tile([128, L["C_total"]], FP16, name="c_dstv",
                              tag="c_dstv")
            nc.sync.dma_start(
                out=dstv_sb[:],
                in_=pi_ap16[:, off["dstv"]:off["dstv"] + L["C_total"]]
                .bitcast(FP16))
            idx_sb = cp.tile([128, L["T_pad"] // 16], I16)
            nc.sync.dma_start(
                out=idx_sb[:],
                in_=pi_ap16[:, off["idx"]:off["idx"] + L["T_pad"] // 16])
            identf = cp.tile([128, 128], F32)
            make_identity(nc, identf[:])
            idf = cp.tile([128, 128], FP16)
            nc.vector.tensor_copy(out=idf[:], in_=identf[:])
            iota8 = cp.tile([128, SB, 128], FP16)
            nc.scalar.dma_start(
                out=iota8[:],
                in_=pi_ap16[:, off["iota"]:off["iota"] + SB * 128]
                .bitcast(FP16).rearrange("p (c f) -> p c f", c=SB))

            # ---- phase A: z1 = dinvA * (x @ W1) -> ag1_in
            # x arrives pre-transposed from the host: lhsT slices directly.
            with (
                tc.tile_pool(name="xt", bufs=1) as xtp,
                tc.tile_pool(name="zs", bufs=8) as zp,
            ):
                xT_sb = xtp.tile([IN_CH, ROWS], F32)
                for k0 in range(0, TPC, 10):
                    k1 = min(k0 + 10, TPC)
                    nc.sync.dma_start(
                        out=xT_sb[:, k0 * 128:k1 * 128],
                        in_=pf_d.ap()[:, off["xT"] + k0 * 128:
                                      off["xT"] + k1 * 128])
                for t in range(TPC):
                    zps = psS.tile([128, 128], F32, space="PSUM",
                                   tag="accS")
                    nc.tensor.matmul(out=zps[:, 0:HID],
                                     lhsT=xT_sb[:, t * 128:(t + 1) * 128],
                                     rhs=w1sb[:], start=True, stop=True)
                    zsb = zp.tile([128, HID], F32, tag="zsb")
                    nc.vector.tensor_scalar(out=zsb[:], in0=zps[:, 0:HID],
                                            scalar1=dinvS_sb[:, t:t + 1],
                                            scalar2=None,
                                            op0=mybir.AluOpType.mult)
                    zcat = zp.tile([128, 2 * HID], FP16, tag="zcat")
                    nc.vector.tensor_copy(out=zcat[:, 0:HID], in_=zsb[:])
                    nc.vector.tensor_tensor(out=zcat[:, HID:2 * HID],
                                            in0=zsb[:],
                                            in1=zcat[:, 0:HID],
                                            op=mybir.AluOpType.subtract)
                    nc.scalar.dma_start(out=ag1_in[t * 128:(t + 1) * 128, :],
                                        in_=zcat[:])

            no_cc = globals().get("_NO_CC", False)
            if prefix >= 2:
                for k in range(R):
                    lo = 128 * CH_START[k]
                    if no_cc:
                        for cc in range(N_CORES):
                            nc.sync.dma_start(
                                out=ag1_out[k][cc * RK[k]:(cc + 1) * RK[k], :],
                                in_=ag1_in[lo:lo + RK[k], :])
                    else:
                        nc.gpsimd.collective_compute(
                            "AllGather", mybir.AluOpType.bypass,
                            replica_groups=[list(range(N_CORES))],
                            ins=[ag1_in[lo:lo + RK[k], :]],
                            outs=[ag1_out[k][:]])

            if prefix >= 3:
                _emit_agg(nc, tc, psS, [a[:] for a in ag1_out], idx_sb, dstv_sb,
                          dinvS_sb, bb1_sb, iota8, L, layer=1,
                          out_d=ag2_in, self_src=ag1_in, idf=idf)

            if prefix >= 4:
                for k in range(R):
                    lo = 128 * CH_START[k]
                    if no_cc:
                        for cc in range(N_CORES):
                            nc.sync.dma_start(
                                out=ag2_out[k][cc * RK[k]:(cc + 1) * RK[k], :],
                                in_=ag2_in[lo:lo + RK[k], :])
                    else:
                        nc.gpsimd.collective_compute(
                            "AllGather", mybir.AluOpType.bypass,
                            replica_groups=[list(range(N_CORES))],
                            ins=[ag2_in[lo:lo + RK[k], :]],
                            outs=[ag2_out[k][:]])

            if prefix >= 5:
                _emit_agg(nc, tc, psS, [a[:] for a in ag2_out], idx_sb, dstv_sb,
                          dinvS_sb, bb1_sb, iota8, L, layer=2,
                          out_d=out_d.ap(), self_src=ag2_in, idf=idf,
                          w2bf=w2bf, bb2_sb=bb2_sb)
    nc.compile()
    return nc


class _SpmdRunner:
    def __init__(self, nc, n_cores=N_CORES):
        bass2jax.install_neuronx_cc_hook()
        self.nc = nc
        self.n_cores = n_cores
        in_names, out_names, out_avals = [], [], []
        partition_name = nc.partition_id_tensor.name if nc.partition_id_tensor \
            else None
        for alloc in nc.m.functions[0].allocations:
            if not isinstance(alloc, mybir.MemoryLocationSet):
                continue
            name = alloc.memorylocations[0].name
            if alloc.kind == "ExternalInput":
                if name != partition_name:
                    in_names.append(name)
            elif alloc.kind == "ExternalOutput":
                out_names.append(name)
                out_avals.append(jax.core.ShapedArray(
                    tuple(alloc.tensor_shape), mybir.dt.np(alloc.dtype)))
        self.in_names, self.out_names, self.out_avals = \
            in_names, out_names, out_avals
        n_params = len(in_names)
        n_outs = len(out_avals)
        all_names = list(in_names) + list(out_names)
        if partition_name is not None:
            all_names.append(partition_name)

        def _body(*args):
            operands = list(args)
            if partition_name is not None:
                operands.append(partition_id_tensor())
            outs = _bass_exec_p.bind(
                *operands,
                out_avals=tuple(out_avals),
                in_names=tuple(all_names),
                out_names=tuple(out_names),
                lowering_input_output_aliases=(),
                sim_require_finite=True,
                sim_require_nnan=True,
                nc=nc,
            )
            return tuple(outs)

        devices = jax.devices()[:n_cores]
        assert len(devices) >= n_cores or len(devices) == n_cores, \
            f"need {n_cores} cores, have {len(jax.devices())}"
        self.mesh = Mesh(np.asarray(devices), ("core",))
        in_specs = (PartitionSpec("core"),) * (n_params + n_outs)
        out_specs = (PartitionSpec("core"),) * n_outs
        self.fn = jax.jit(
            shard_map(_body, mesh=self.mesh, in_specs=in_specs,
                      out_specs=out_specs, check_rep=False),
            keep_unused=True,
        )

    def run(self, in_maps):
        concat_in = [
            np.concatenate([np.asarray(in_maps[c][nm])
                            for c in range(self.n_cores)], axis=0)
            for nm in self.in_names
        ]
        concat_zeros = [
            np.zeros((self.n_cores * av.shape[0], *av.shape[1:]), av.dtype)
            for av in self.out_avals
        ]
        outs = self.fn(*(concat_in + concat_zeros))
        jax.block_until_ready(outs)
        res = []
        for c in range(self.n_cores):
            d = {}
            for i, nm in enumerate(self.out_names):
                a = np.asarray(outs[i]).reshape(self.n_cores,
                                                *self.out_avals[i].shape)
                d[nm] = a[c]
            res.append(d)
        return res


_CACHE = {}


def _get_programs(edge_index):
    key = hash(np.asarray(edge_index).tobytes())
    if key not in _CACHE:
        pre = _preprocess(edge_index)
        ncM = _build_merged(pre)
        _CACHE[key] = (pre, _SpmdRunner(ncM))
    return _CACHE[key]


def _make_maps(pre, x, W1, b1, W2, b2):
    off = pre["off"]
    tile_of = pre["tile_of"]
    bb1 = np.tile(b1, (128, 1)).astype(np.float32)
    bb2 = np.tile(b2, (128, 1)).astype(np.float32)
    w2pad = np.zeros((128, OUT_CH), np.float32)
    w2pad[:HID] = W2
    maps = []
    for c in range(N_CORES):
        xs = np.zeros((ROWS, IN_CH), np.float32)
        for t in range(TPC):
            tl = tile_of[c, t]
            if tl >= 0:
                xs[t * 128:(t + 1) * 128] = x[tl * 128:(tl + 1) * 128]
        pf = np.empty((128, off["_ftotal"]), np.float32)
        pf[:, off["xT"]:off["xT"] + ROWS] = xs.T
        pf[:, off["w1"]:off["w1"] + HID] = W1
        pf[:, off["w2"]:off["w2"] + OUT_CH] = w2pad
        pf[:, off["dinvS"]:off["dinvS"] + TPC] = pre["dinvS"][c]
        pf[:, off["bb1"]:off["bb1"] + HID] = bb1
        pf[:, off["bb2"]:off["bb2"] + OUT_CH] = bb2
        pi = np.zeros((128, off["_itotal"]), np.int16)
        pi[:, :off["dstv"]] = pre["L"]["idxw"][c]
        pi[:, off["dstv"]:off["dstv"] + pre["L"]["C_total"]] = \
            pre["L"]["dstv"][c].astype(np.float16).view(np.int16)
        iota = np.tile(np.arange(128, dtype=np.float16), SB)
        pi[:, off["iota"]:off["iota"] + SB * 128] = \
            np.tile(iota.view(np.int16), (128, 1))
        maps.append({"pf": np.concatenate([pf, pi.view(np.float32)],
                                          axis=1)})
    return maps


def kernel(x, edge_index, W1, b1, W2, b2):
    x = np.asarray(x, np.float32)
    W1 = np.asarray(W1, np.float32)
    b1 = np.asarray(b1, np.float32)
    W2 = np.asarray(W2, np.float32)
    b2 = np.asarray(b2, np.float32)
    pre, rM = _get_programs(edge_index)
    maps = _make_maps(pre, x, W1, b1, W2, b2)
    res = rM.run(maps)
    tile_of = pre["tile_of"]
    out = np.zeros((N_NODES, OUT_CH), np.float32)
    for c in range(N_CORES):
        o = np.asarray(res[c]["outp"], np.float32)
        for t in range(TPC):
            tl = tile_of[c, t]
            if tl >= 0:
                out[tl * 128:(tl + 1) * 128] = o[t * 128:(t + 1) * 128]
    return out
